# revision 8
# baseline (speedup 1.0000x reference)
"""Trainium2 Bass kernel for nn_AttentionOutputModule (sparse attention + MLPs).

Sharding: 8 cores, each owns 2500 destination rows; edges partitioned by
destination row. Per core, rows are sorted by degree and grouped into 128-row
blocks; each block's edge lists are padded to the block max degree. k (for all
20000 nodes) + pos are written to a DRAM table, and per-edge rows are fetched
with dma_gather. Softmax is computed without max-subtraction (logits are
bounded, mathematically identical). MLPs run in a transposed orientation
(dims on partitions) so no per-layer activation transposes are needed.
"""
import functools
import numpy as np
import ml_dtypes

import concourse.bacc as bacc
import concourse.mybir as mybir
from concourse import tile, bass_utils

# problem dims (hardcoded per contract)
N, DM, H, HD, HID, FIN = 20000, 512, 8, 64, 1024, 536
NCORES = 8
RPC = N // NCORES          # 2500 rows per core
RPAD = 2560                # padded to 20 blocks of 128
NBLK = RPAD // 128         # 20
GRP = 4                    # blocks per MLP group (512 rows)
NGRP = NBLK // GRP         # 5
JCH = 8                    # j's per gather chunk -> 1024 idxs
TB = (N + 127) // 128      # 157 x_full blocks
TROWS = TB * 128           # 20096
TC = 576                   # gather table cols: k(512) | pos(3) | pad
SCALE = 1.0 / float(np.sqrt(HD))
EPS = 1e-5
PADBIAS = -30000.0

F32 = mybir.dt.float32
BF16 = mybir.dt.bfloat16
I16 = mybir.dt.int16
AF = mybir.ActivationFunctionType
OP = mybir.AluOpType
AX = mybir.AxisListType


def _layernorm_block(nc, pool, xt, tag):
    """LN_raw of [128, 512] fp32 tile -> bf16 tile (affine folded into weights)."""
    s1 = pool.tile([128, 1], F32, tag=f"{tag}_s1")
    nc.vector.tensor_reduce(s1[:], xt[:], axis=AX.X, op=OP.add)
    sq = pool.tile([128, DM], F32, tag=f"{tag}_sq")
    s2 = pool.tile([128, 1], F32, tag=f"{tag}_s2")
    nc.scalar.activation(sq[:], xt[:], AF.Square, accum_out=s2[:])
    m = pool.tile([128, 1], F32, tag=f"{tag}_m")
    nc.vector.tensor_scalar_mul(m[:], s1[:], 1.0 / DM)
    m2 = pool.tile([128, 1], F32, tag=f"{tag}_m2")
    nc.vector.tensor_tensor(m2[:], m[:], m[:], op=OP.mult)
    t2 = pool.tile([128, 1], F32, tag=f"{tag}_t2")
    nc.vector.tensor_scalar(t2[:], s2[:], 1.0 / DM, EPS, op0=OP.mult, op1=OP.add)
    var = pool.tile([128, 1], F32, tag=f"{tag}_var")
    nc.vector.tensor_tensor(var[:], t2[:], m2[:], op=OP.subtract)
    sd = pool.tile([128, 1], F32, tag=f"{tag}_sd")
    nc.scalar.activation(sd[:], var[:], AF.Sqrt)
    inv = pool.tile([128, 1], F32, tag=f"{tag}_inv")
    nc.vector.reciprocal(inv[:], sd[:])
    zb = pool.tile([128, DM], BF16, tag=f"{tag}_zb")
    nc.vector.tensor_scalar(zb[:], xt[:], m[:], inv[:],
                            op0=OP.subtract, op1=OP.mult)
    return zb


def _transpose128(nc, pool, psp, src_ap, ident, tag, out_dtype=BF16):
    """PE-transpose a [128,128] bf16 AP -> sbuf tile [128,128]."""
    tp = psp.tile([128, 128], BF16, tag="psB")
    nc.tensor.transpose(tp[:], src_ap, ident[:])
    ts = pool.tile([128, 128], out_dtype, tag=f"{tag}_sb")
    nc.scalar.activation(ts[:], tp[:], AF.Copy)
    return ts


@functools.lru_cache(maxsize=2)
def _build(dbs: tuple, total_chunks: int, totb: int):
    """Build + compile the SPMD program. dbs: padded per-block degrees (len 20)."""
    nc = bacc.Bacc("TRN2", target_bir_lowering=False, debug=False,
                   num_devices=NCORES)

    def inp(nm, sh, dt=F32):
        return nc.dram_tensor(nm, sh, dt, kind="ExternalInput").ap()

    xf = inp("xf", [TROWS, DM])
    xp = inp("xp", [RPAD, DM])
    posf = inp("posf", [N, 3])
    posp = inp("posp", [RPAD, 3])
    colidx = inp("colidx", [total_chunks, 128, JCH * 128 // 16], I16)
    biasfl = inp("biasfl", [totb])
    ident_in = inp("ident", [128, 128], BF16)
    wq_in = inp("wq", [4, 128, DM], BF16)
    wk_in = inp("wk", [4, 128, DM], BF16)
    wei_in = inp("wei", [4, 128, HID], BF16)
    weh_in = inp("weh", [8, 128, HID], BF16)
    weo_in = inp("weo", [8, 128, 1], BF16)
    wfi_in = inp("wfi", [5, 128, HID], BF16)
    wfh_in = inp("wfh", [8, 128, HID], BF16)
    wfo_in = inp("wfo", [8, 128, 3], BF16)
    out = nc.dram_tensor("out", [NGRP, 4, 512], F32, kind="ExternalOutput").ap()

    with tile.TileContext(nc) as tc:
        with tc.tile_pool(name="dram", bufs=1, space="DRAM") as dpool, \
             tc.tile_pool(name="wpool", bufs=1) as wp, \
             tc.tile_pool(name="pool", bufs=2) as pool, \
             tc.tile_pool(name="gpool", bufs=2) as gp, \
             tc.tile_pool(name="mpool", bufs=1) as mp, \
             tc.tile_pool(name="psum", bufs=2, space="PSUM") as psp:

            ktab = dpool.tile([TROWS, TC], F32)

            # resident weights
            def load_w(ap_in, nchunk, ncol, nm):
                t = wp.tile([128, nchunk, ncol], BF16, tag=nm)
                nc.sync.dma_start(t[:], ap_in.transpose([1, 0, 2]))
                return t

            wq = load_w(wq_in, 4, DM, "wq")
            wk = load_w(wk_in, 4, DM, "wk")
            wei = load_w(wei_in, 4, HID, "wei")
            weh = load_w(weh_in, 8, HID, "weh")
            weo = load_w(weo_in, 8, 1, "weo")
            wfi = load_w(wfi_in, 5, HID, "wfi")
            wfh = load_w(wfh_in, 8, HID, "wfh")
            wfo = load_w(wfo_in, 8, 3, "wfo")
            ident = wp.tile([128, 128], BF16, tag="ident")
            nc.sync.dma_start(ident[:], ident_in)

            # ---- phase 1: k table (all 20096 rows, replicated on each core)
            nc.sync.dma_start(ktab[0:N, 512:515], posf)
            for tb in range(TB):
                xt = pool.tile([128, DM], F32, tag="p1_x")
                nc.sync.dma_start(xt[:], xf[tb * 128:(tb + 1) * 128, :])
                zb = _layernorm_block(nc, pool, xt, "p1")
                kp = psp.tile([128, DM], F32, tag="psA")
                for c in range(4):
                    zt = _transpose128(nc, pool, psp,
                                       zb[:, c * 128:(c + 1) * 128], ident,
                                       "p1_zt")
                    nc.tensor.matmul(kp[:], zt[:], wk[:, c, :],
                                     start=(c == 0), stop=(c == 3))
                ks = pool.tile([128, DM], F32, tag="p1_ks")
                nc.scalar.activation(ks[:], kp[:], AF.Copy)
                nc.sync.dma_start(ktab[tb * 128:(tb + 1) * 128, 0:512], ks[:])

            # ---- phase 2+3: per 512-row group
            chunk_ctr = 0
            bias_off = 0
            for g in range(NGRP):
                zTg = pool.tile([128, GRP, 4, 128], BF16, tag="zTg")
                at5 = pool.tile([128, GRP, 128], BF16, tag="at5")
                for sb in range(GRP):
                    b = g * GRP + sb
                    Db = dbs[b]
                    xt = pool.tile([128, DM], F32, tag="p2_x")
                    nc.sync.dma_start(xt[:], xp[b * 128:(b + 1) * 128, :])
                    zb = _layernorm_block(nc, pool, xt, "p2")
                    for c in range(4):
                        tp = psp.tile([128, 128], BF16, tag="psB")
                        nc.tensor.transpose(tp[:], zb[:, c * 128:(c + 1) * 128],
                                            ident[:])
                        nc.scalar.activation(zTg[:, sb, c, :], tp[:], AF.Copy)
                    # q = z @ Wq  [128 rows, 512] fp32
                    qp = psp.tile([128, DM], F32, tag="psA")
                    for c in range(4):
                        nc.tensor.matmul(qp[:], zTg[:, sb, c, :], wq[:, c, :],
                                         start=(c == 0), stop=(c == 3))
                    qs = pool.tile([128, DM], F32, tag="qs")
                    nc.scalar.activation(qs[:], qp[:], AF.Copy)

                    # attention
                    A = pool.tile([128, H, 3], F32, tag="A")
                    if Db > 0:
                        L = pool.tile([128, H, Db], F32, tag="L")
                        posb = pool.tile([128, Db, 3], F32, tag="posb")
                        nchunk = Db // JCH
                        for ch in range(nchunk):
                            idxt = gp.tile([128, JCH * 128 // 16], I16,
                                           tag="idxt")
                            nc.sync.dma_start(idxt[:], colidx[chunk_ctr])
                            chunk_ctr += 1
                            G = gp.tile([128, JCH, TC], F32, tag="G")
                            nc.gpsimd.dma_gather(
                                G[:], ktab[:], idxt[:],
                                num_idxs=JCH * 128, num_idxs_reg=JCH * 128,
                                elem_size=TC)
                            nc.vector.tensor_copy(
                                posb[:, ch * JCH:(ch + 1) * JCH, :],
                                G[:, :, 512:515])
                            for h in range(H):
                                pr = pool.tile([128, JCH, HD], F32, tag="pr")
                                nc.vector.tensor_tensor(
                                    pr[:], G[:, :, h * HD:(h + 1) * HD],
                                    qs[:, h * HD:(h + 1) * HD].unsqueeze(1)
                                      .broadcast_to([128, JCH, HD]),
                                    op=OP.mult)
                                nc.vector.tensor_reduce(
                                    L[:, h, ch * JCH:(ch + 1) * JCH], pr[:],
                                    axis=AX.X, op=OP.add)
                        # logits -> softmax weights
                        bt = pool.tile([128, Db], F32, tag="bt")
                        nc.sync.dma_start(
                            bt[:],
                            biasfl[bias_off:bias_off + 128 * Db]
                            .rearrange("(p d) -> p d", p=128))
                        bias_off += 128 * Db
                        nc.vector.scalar_tensor_tensor(
                            L[:], L[:], SCALE,
                            bt[:].unsqueeze(1).broadcast_to([128, H, Db]),
                            op0=OP.mult, op1=OP.add)
                        P = pool.tile([128, H, Db], F32, tag="P")
                        nc.scalar.activation(P[:], L[:], AF.Exp)
                        dn = pool.tile([128, H], F32, tag="dn")
                        nc.vector.tensor_reduce(dn[:], P[:], axis=AX.X,
                                                op=OP.add)
                        nc.vector.tensor_scalar_max(dn[:], dn[:], 1e-35)
                        rc = pool.tile([128, H], F32, tag="rc")
                        nc.vector.reciprocal(rc[:], dn[:])
                        nc.vector.tensor_tensor(
                            P[:], P[:],
                            rc[:].unsqueeze(2).broadcast_to([128, H, Db]),
                            op=OP.mult)
                        for cc in range(3):
                            tt = pool.tile([128, H, Db], F32, tag="tt")
                            nc.vector.tensor_tensor(
                                tt[:], P[:],
                                posb[:, :, cc].unsqueeze(1)
                                .broadcast_to([128, H, Db]),
                                op=OP.mult)
                            nc.vector.tensor_reduce(A[:, :, cc], tt[:],
                                                    axis=AX.X, op=OP.add)
                    else:
                        nc.vector.memset(A[:], 0.0)
                    # att -= pos
                    pp = pool.tile([128, 3], F32, tag="pp")
                    nc.sync.dma_start(pp[:], posp[b * 128:(b + 1) * 128, :])
                    nc.vector.tensor_tensor(
                        A[:], A[:],
                        pp[:].unsqueeze(1).broadcast_to([128, H, 3]),
                        op=OP.subtract)
                    # A^T padded into at5[:, sb, :]
                    Ab = pool.tile([128, 24], BF16, tag="Ab")
                    nc.scalar.activation(Ab[:], A[:].rearrange("p h c -> p (h c)"),
                                         AF.Copy)
                    atp = psp.tile([128, 128], BF16, tag="psB")
                    nc.tensor.transpose(atp[0:24, :], Ab[:], ident[:])
                    nc.vector.memset(at5[:, sb, :], 0.0)
                    nc.scalar.activation(at5[0:24, sb, :], atp[0:24, :], AF.Copy)

                # ---- MLPs for this 512-row group (transposed orientation)
                def rhs_z(ic):
                    return zTg[:, :, ic, :]

                def mlp_layer(win, nin, rhs_fn, nm, gelu=True):
                    ht = mp.tile([128, 8, 512], BF16, tag=nm)
                    for oc in range(8):
                        ps = psp.tile([128, 512], F32, tag="psA")
                        for ic in range(nin):
                            nc.tensor.matmul(ps[:], win[:, ic, oc * 128:(oc + 1) * 128],
                                             rhs_fn(ic), start=(ic == 0),
                                             stop=(ic == nin - 1))
                        if gelu:
                            nc.scalar.activation(ht[:, oc, :], ps[:],
                                                 AF.Gelu_apprx_tanh)
                        else:
                            nc.scalar.activation(ht[:, oc, :], ps[:], AF.Copy)
                    return ht

                # energy
                h1e = mlp_layer(wei, 4, rhs_z, "h1e")
                g2e = mlp_layer(weh, 8, lambda ic: h1e[:, ic, :], "g2e")
                h2e = mp.tile([128, 8, 512], BF16, tag="h2e")
                nc.vector.tensor_tensor(h2e[:], g2e[:], h1e[:], op=OP.add)
                pet = psp.tile([3, 512], F32, tag="psC")
                pe = pet[0:1, :]
                for ic in range(8):
                    nc.tensor.matmul(pe[:], weo[:, ic, :], h2e[:, ic, :],
                                     start=(ic == 0), stop=(ic == 7))
                es = pool.tile([1, 512], F32, tag="es")
                nc.scalar.activation(es[:], pe[:], AF.Copy)

                # forces
                def rhs_f(ic):
                    if ic < 4:
                        return rhs_z(ic)
                    return at5[:]

                h1f = mlp_layer(wfi, 5, rhs_f, "h1f")
                g2f = mlp_layer(wfh, 8, lambda ic: h1f[:, ic, :], "g2f")
                h2f = mp.tile([128, 8, 512], BF16, tag="h2f")
                nc.vector.tensor_tensor(h2f[:], g2f[:], h1f[:], op=OP.add)
                pf = psp.tile([3, 512], F32, tag="psC")
                for ic in range(8):
                    nc.tensor.matmul(pf[:], wfo[:, ic, :], h2f[:, ic, :],
                                     start=(ic == 0), stop=(ic == 7))
                fs = pool.tile([3, 512], F32, tag="fs")
                nc.scalar.activation(fs[:], pf[:], AF.Copy)
                nc.sync.dma_start(out[g, 0:1, :], es[:])
                nc.sync.dma_start(out[g, 1:4, :], fs[:])

    nc.compile()
    return nc


def _prep(inputs):
    """Host-side sharding: returns (in_maps, perms, dbs, total_chunks, totb)."""
    x = np.asarray(inputs["x"], np.float32)
    ei = np.asarray(inputs["edge_index"]).astype(np.int64)
    ab = np.asarray(inputs["att_bias"], np.float32)
    pos = np.asarray(inputs["pos"], np.float32)

    row, col = ei[0], ei[1]
    order = np.argsort(row, kind="stable")
    rs, cs, bs = row[order], col[order], ab[order]
    core_bounds = np.searchsorted(rs, np.arange(NCORES + 1) * RPC)

    # per-core degree & permutation
    degs, perms = [], []
    for m in range(NCORES):
        lo, hi = core_bounds[m], core_bounds[m + 1]
        deg = np.bincount(rs[lo:hi] - m * RPC, minlength=RPC)
        perm = np.argsort(-deg, kind="stable")
        degs.append(deg)
        perms.append(perm)

    # global (cross-core max) padded block degrees
    dbs = []
    for b in range(NBLK):
        mx = 0
        for m in range(NCORES):
            blk = perms[m][b * 128:(b + 1) * 128]
            if len(blk):
                mx = max(mx, int(degs[m][blk].max()) if b * 128 < RPC else 0)
        mx = max(mx, JCH)
        dbs.append(((mx + JCH - 1) // JCH) * JCH)
    dbs = tuple(dbs)
    total_chunks = sum(d // JCH for d in dbs)
    totb = 128 * sum(dbs)

    # folded weights (fp32 host math)
    g_att = np.asarray(inputs["g_att"], np.float32)
    b_att = np.asarray(inputs["b_att"], np.float32)
    g_mlp = np.asarray(inputs["g_mlp"], np.float32)
    b_mlp = np.asarray(inputs["b_mlp"], np.float32)
    Wq = np.asarray(inputs["Wq"], np.float32) * g_att[:, None]
    Wk = np.asarray(inputs["Wk"], np.float32) * g_att[:, None]
    Wei = np.asarray(inputs["We_in"], np.float32) * g_mlp[:, None]
    Wfi = np.asarray(inputs["Wf_in"], np.float32).copy()
    Wfi[:512] *= g_mlp[:, None]
    for nm, bsrc, wmat in [("bq", "bq", None)]:
        pass
    bias_names = ["bq", "bk", "be_in", "be_h", "be_out", "bf_in", "bf_h",
                  "bf_out"]
    allb = [np.asarray(inputs[n], np.float32) for n in bias_names]
    bq2 = allb[0] + b_att @ np.asarray(inputs["Wq"], np.float32)
    bk2 = allb[1] + b_att @ np.asarray(inputs["Wk"], np.float32)
    bei2 = allb[2] + b_mlp @ np.asarray(inputs["We_in"], np.float32)
    bfi2 = allb[5] + b_mlp @ np.asarray(inputs["Wf_in"], np.float32)[:512]
    assert all(np.abs(v).max() == 0 for v in
               [bq2, bk2, bei2, bfi2, allb[3], allb[4], allb[6], allb[7]]), \
        "nonzero biases not supported by this build"

    Weh = np.asarray(inputs["We_h"], np.float32)
    Weo = np.asarray(inputs["We_out"], np.float32)
    Wfh = np.asarray(inputs["Wf_h"], np.float32)
    Wfo = np.asarray(inputs["Wf_out"], np.float32)

    def wtile(W, nchunk):
        Wp = np.zeros((nchunk * 128, W.shape[1]), np.float32)
        Wp[:W.shape[0]] = W
        return Wp.reshape(nchunk, 128, W.shape[1]).astype(ml_dtypes.bfloat16)

    xf_np = np.zeros((TROWS, DM), np.float32)
    xf_np[:N] = x
    shared = {
        "xf": xf_np, "posf": pos,
        "ident": np.eye(128).astype(ml_dtypes.bfloat16),
        "wq": wtile(Wq, 4), "wk": wtile(Wk, 4),
        "wei": wtile(Wei, 4), "weh": wtile(Weh, 8), "weo": wtile(Weo, 8),
        "wfi": wtile(Wfi, 5), "wfh": wtile(Wfh, 8), "wfo": wtile(Wfo, 8),
    }

    in_maps = []
    for m in range(NCORES):
        lo = core_bounds[m]
        deg, perm = degs[m], perms[m]
        perm_pad = np.concatenate([perm, np.zeros(RPAD - RPC, np.int64)])
        deg_pad = np.concatenate([deg[perm], np.zeros(RPAD - RPC, np.int64)])
        # edge offsets into the sorted-per-core arrays
        roff = np.concatenate([[0], np.cumsum(deg)]) + lo

        colchunks = np.zeros((total_chunks, JCH * 128), np.int16)
        biasfl = np.full(totb, PADBIAS, np.float32)
        ci = 0
        boff = 0
        for b in range(NBLK):
            Db = dbs[b]
            cols_blk = np.zeros((128, Db), np.int64)
            bias_blk = np.full((128, Db), PADBIAS, np.float32)
            for p in range(128):
                r = perm_pad[b * 128 + p]
                d = int(deg_pad[b * 128 + p])
                if d:
                    e0 = roff[r]
                    cols_blk[p, :d] = cs[e0:e0 + d]
                    bias_blk[p, :d] = bs[e0:e0 + d]
            biasfl[boff:boff + 128 * Db] = bias_blk.reshape(-1)
            boff += 128 * Db
            for ch in range(Db // JCH):
                # gather order: position jj*128 + p
                colchunks[ci] = cols_blk[:, ch * JCH:(ch + 1) * JCH].T.reshape(-1)
                ci += 1
        # wrap indices: idx i -> [i%16, i//16], replicate x8
        cw = colchunks.reshape(total_chunks, JCH * 128 // 16, 16)
        cw = np.ascontiguousarray(np.transpose(cw, (0, 2, 1)))
        cidx = np.tile(cw, (1, 8, 1)).astype(np.int16)

        im = dict(shared)
        im["xp"] = x[m * RPC:(m + 1) * RPC][perm_pad % RPC]
        im["posp"] = pos[m * RPC:(m + 1) * RPC][perm_pad % RPC]
        im["colidx"] = cidx
        im["biasfl"] = biasfl
        in_maps.append(im)
    return in_maps, perms, dbs, total_chunks, totb


def kernel(**inputs):
    in_maps, perms, dbs, total_chunks, totb = _prep(inputs)
    nc = _build(dbs, total_chunks, totb)
    res = bass_utils.run_bass_kernel_spmd(
        nc, in_maps, core_ids=list(range(NCORES)))
    energy = np.zeros((N, 1), np.float32)
    forces = np.zeros((N, 3), np.float32)
    for m in range(NCORES):
        o = res.results[m]["out"]              # [NGRP, 4, 512]
        e_perm = o[:, 0, :].reshape(RPAD)
        f_perm = np.moveaxis(o[:, 1:4, :], 1, 2).reshape(RPAD, 3)
        gi = m * RPC + perms[m]
        energy[gi, 0] = e_perm[:RPC]
        forces[gi] = f_perm[:RPC]
    return energy, forces


# revision 9
# speedup vs baseline: 1.0124x; 1.0124x over previous
"""Trainium2 Bass kernel for nn_AttentionOutputModule (sparse attention + MLPs).

Sharding: 8 cores, each owns 2500 destination rows; edges partitioned by
destination row. Per core, rows are sorted by degree and grouped into 128-row
blocks; each block's edge lists are padded to the block max degree. k (for all
20000 nodes) + pos are written to a DRAM table, and per-edge rows are fetched
with dma_gather. Softmax is computed without max-subtraction (logits are
bounded, mathematically identical). MLPs run in a transposed orientation
(dims on partitions) so no per-layer activation transposes are needed.
"""
import functools
import numpy as np
import ml_dtypes

import concourse.bacc as bacc
import concourse.mybir as mybir
from concourse import tile, bass_utils

# problem dims (hardcoded per contract)
N, DM, H, HD, HID, FIN = 20000, 512, 8, 64, 1024, 536
NCORES = 8
RPC = N // NCORES          # 2500 rows per core
RPAD = 2560                # padded to 20 blocks of 128
NBLK = RPAD // 128         # 20
GRP = 4                    # blocks per MLP group (512 rows)
NGRP = NBLK // GRP         # 5
JCH = 8                    # j's per gather chunk -> 1024 idxs
TB = (N + 127) // 128      # 157 x_full blocks
TROWS = TB * 128           # 20096
TC = 640                   # bf16 gather table cols: k(512)bf16 | pos(3)f32-bits | pad
SCALE = 1.0 / float(np.sqrt(HD))
EPS = 1e-5
PADBIAS = -30000.0

F32 = mybir.dt.float32
BF16 = mybir.dt.bfloat16
I16 = mybir.dt.int16
AF = mybir.ActivationFunctionType
OP = mybir.AluOpType
AX = mybir.AxisListType


def _layernorm_block(nc, pool, xt, tag):
    """LN_raw of [128, 512] fp32 tile -> bf16 tile (affine folded into weights)."""
    s1 = pool.tile([128, 1], F32, tag=f"{tag}_s1")
    nc.vector.tensor_reduce(s1[:], xt[:], axis=AX.X, op=OP.add)
    sq = pool.tile([128, DM], F32, tag=f"{tag}_sq")
    s2 = pool.tile([128, 1], F32, tag=f"{tag}_s2")
    nc.scalar.activation(sq[:], xt[:], AF.Square, accum_out=s2[:])
    m = pool.tile([128, 1], F32, tag=f"{tag}_m")
    nc.vector.tensor_scalar_mul(m[:], s1[:], 1.0 / DM)
    m2 = pool.tile([128, 1], F32, tag=f"{tag}_m2")
    nc.vector.tensor_tensor(m2[:], m[:], m[:], op=OP.mult)
    t2 = pool.tile([128, 1], F32, tag=f"{tag}_t2")
    nc.vector.tensor_scalar(t2[:], s2[:], 1.0 / DM, EPS, op0=OP.mult, op1=OP.add)
    var = pool.tile([128, 1], F32, tag=f"{tag}_var")
    nc.vector.tensor_tensor(var[:], t2[:], m2[:], op=OP.subtract)
    sd = pool.tile([128, 1], F32, tag=f"{tag}_sd")
    nc.scalar.activation(sd[:], var[:], AF.Sqrt)
    inv = pool.tile([128, 1], F32, tag=f"{tag}_inv")
    nc.vector.reciprocal(inv[:], sd[:])
    zb = pool.tile([128, DM], BF16, tag=f"{tag}_zb")
    nc.vector.tensor_scalar(zb[:], xt[:], m[:], inv[:],
                            op0=OP.subtract, op1=OP.mult)
    return zb


def _transpose128(nc, pool, psp, src_ap, ident, tag, out_dtype=BF16):
    """PE-transpose a [128,128] bf16 AP -> sbuf tile [128,128]."""
    tp = psp.tile([128, 128], BF16, tag="psB")
    nc.tensor.transpose(tp[:], src_ap, ident[:])
    ts = pool.tile([128, 128], out_dtype, tag=f"{tag}_sb")
    nc.scalar.activation(ts[:], tp[:], AF.Copy)
    return ts


@functools.lru_cache(maxsize=2)
def _build(dbs: tuple, total_chunks: int, totb: int):
    """Build + compile the SPMD program. dbs: padded per-block degrees (len 20)."""
    nc = bacc.Bacc("TRN2", target_bir_lowering=False, debug=False,
                   num_devices=NCORES)

    def inp(nm, sh, dt=F32):
        return nc.dram_tensor(nm, sh, dt, kind="ExternalInput").ap()

    xf = inp("xf", [TROWS, DM])
    xp = inp("xp", [RPAD, DM])
    posf = inp("posf", [N, 3])
    posp = inp("posp", [RPAD, 3])
    colidx = inp("colidx", [total_chunks, 128, JCH * 128 // 16], I16)
    biasfl = inp("biasfl", [totb])
    ident_in = inp("ident", [128, 128], BF16)
    wq_in = inp("wq", [4, 128, DM], BF16)
    wk_in = inp("wk", [4, 128, DM], BF16)
    wei_in = inp("wei", [4, 128, HID], BF16)
    weh_in = inp("weh", [8, 128, HID], BF16)
    weo_in = inp("weo", [8, 128, 1], BF16)
    wfi_in = inp("wfi", [5, 128, HID], BF16)
    wfh_in = inp("wfh", [8, 128, HID], BF16)
    wfo_in = inp("wfo", [8, 128, 3], BF16)
    out = nc.dram_tensor("out", [NGRP, 4, 512], F32, kind="ExternalOutput").ap()

    with tile.TileContext(nc) as tc:
        with tc.tile_pool(name="dram", bufs=1, space="DRAM") as dpool, \
             tc.tile_pool(name="wpool", bufs=1) as wp, \
             tc.tile_pool(name="pool", bufs=2) as pool, \
             tc.tile_pool(name="gpool", bufs=2) as gp, \
             tc.tile_pool(name="mpool", bufs=1) as mp, \
             tc.tile_pool(name="psum", bufs=2, space="PSUM") as psp:

            ktab = dpool.tile([TROWS, TC], BF16)

            # resident weights
            def load_w(ap_in, nchunk, ncol, nm):
                t = wp.tile([128, nchunk, ncol], BF16, tag=nm)
                nc.sync.dma_start(t[:], ap_in.transpose([1, 0, 2]))
                return t

            wq = load_w(wq_in, 4, DM, "wq")
            wk = load_w(wk_in, 4, DM, "wk")
            wei = load_w(wei_in, 4, HID, "wei")
            weh = load_w(weh_in, 8, HID, "weh")
            weo = load_w(weo_in, 8, 1, "weo")
            wfi = load_w(wfi_in, 5, HID, "wfi")
            wfh = load_w(wfh_in, 8, HID, "wfh")
            wfo = load_w(wfo_in, 8, 3, "wfo")
            ident = wp.tile([128, 128], BF16, tag="ident")
            nc.sync.dma_start(ident[:], ident_in)

            # ---- phase 1: k table (all 20096 rows, replicated on each core)
            nc.sync.dma_start(ktab[:, :].bitcast(F32)[0:N, 256:259], posf)
            identf = wp.tile([128, 128], F32, tag="identf")
            nc.vector.tensor_copy(identf[:], ident[:])
            for tb in range(TB):
                xt = pool.tile([128, DM], F32, tag="p1_x")
                nc.sync.dma_start(xt[:], xf[tb * 128:(tb + 1) * 128, :])
                # LN stats only (mean folded into wk'' on host)
                s1 = pool.tile([128, 1], F32, tag="p1_s1")
                nc.vector.tensor_reduce(s1[:], xt[:], axis=AX.X, op=OP.add)
                sq = pool.tile([128, DM], F32, tag="p1_sq")
                s2 = pool.tile([128, 1], F32, tag="p1_s2")
                nc.scalar.activation(sq[:], xt[:], AF.Square, accum_out=s2[:])
                m = pool.tile([128, 1], F32, tag="p1_m")
                nc.vector.tensor_scalar_mul(m[:], s1[:], 1.0 / DM)
                m2 = pool.tile([128, 1], F32, tag="p1_m2")
                nc.vector.tensor_tensor(m2[:], m[:], m[:], op=OP.mult)
                t2 = pool.tile([128, 1], F32, tag="p1_t2")
                nc.vector.tensor_scalar(t2[:], s2[:], 1.0 / DM, EPS,
                                        op0=OP.mult, op1=OP.add)
                var = pool.tile([128, 1], F32, tag="p1_var")
                nc.vector.tensor_tensor(var[:], t2[:], m2[:], op=OP.subtract)
                sd = pool.tile([128, 1], F32, tag="p1_sd")
                nc.scalar.activation(sd[:], var[:], AF.Sqrt)
                inv = pool.tile([128, 1], F32, tag="p1_inv")
                nc.vector.reciprocal(inv[:], sd[:])
                kp = psp.tile([128, DM], F32, tag="psA")
                for c in range(4):
                    tp = psp.tile([128, 128], F32, tag="psB")
                    nc.tensor.transpose(tp[:], xt[:, c * 128:(c + 1) * 128],
                                        identf[:])
                    xT = pool.tile([128, 128], BF16, tag="p1_xT")
                    nc.any.tensor_copy(xT[:], tp[:])
                    nc.tensor.matmul(kp[:], xT[:], wk[:, c, :],
                                     start=(c == 0), stop=(c == 3))
                ks = pool.tile([128, DM], BF16, tag="p1_ks")
                nc.scalar.activation(ks[:], kp[:], AF.Copy, scale=inv[:])
                nc.sync.dma_start(ktab[tb * 128:(tb + 1) * 128, 0:512], ks[:])

            # ---- phase 2+3: per 512-row group
            chunk_ctr = 0
            bias_off = 0
            for g in range(NGRP):
                zTg = pool.tile([128, GRP, 4, 128], BF16, tag="zTg")
                at5 = pool.tile([128, GRP, 128], BF16, tag="at5")
                for sb in range(GRP):
                    b = g * GRP + sb
                    Db = dbs[b]
                    xt = pool.tile([128, DM], F32, tag="p2_x")
                    nc.sync.dma_start(xt[:], xp[b * 128:(b + 1) * 128, :])
                    zb = _layernorm_block(nc, pool, xt, "p2")
                    for c in range(4):
                        tp = psp.tile([128, 128], BF16, tag="psB")
                        nc.tensor.transpose(tp[:], zb[:, c * 128:(c + 1) * 128],
                                            ident[:])
                        nc.scalar.activation(zTg[:, sb, c, :], tp[:], AF.Copy)
                    # q = z @ Wq  [128 rows, 512] fp32
                    qp = psp.tile([128, DM], F32, tag="psA")
                    for c in range(4):
                        nc.tensor.matmul(qp[:], zTg[:, sb, c, :], wq[:, c, :],
                                         start=(c == 0), stop=(c == 3))
                    qs = pool.tile([128, DM], BF16, tag="qs")
                    nc.scalar.activation(qs[:], qp[:], AF.Copy)

                    # attention
                    A = pool.tile([128, H, 3], F32, tag="A")
                    if Db > 0:
                        L = pool.tile([128, Db, H], F32, tag="L")
                        posb = pool.tile([128, Db, 3], F32, tag="posb")
                        nchunk = Db // JCH
                        for ch in range(nchunk):
                            idxt = gp.tile([128, JCH * 128 // 16], I16,
                                           tag="idxt")
                            nc.sync.dma_start(idxt[:], colidx[chunk_ctr])
                            chunk_ctr += 1
                            G = gp.tile([128, JCH, TC], BF16, tag="G")
                            nc.gpsimd.dma_gather(
                                G[:], ktab[:], idxt[:],
                                num_idxs=JCH * 128, num_idxs_reg=JCH * 128,
                                elem_size=TC)
                            nc.vector.tensor_copy(
                                posb[:, ch * JCH:(ch + 1) * JCH, :],
                                G[:, :, 512:518].bitcast(F32))
                            prod = pool.tile([128, JCH, 512], BF16, tag="prod")
                            nc.vector.tensor_tensor(
                                prod[:], G[:, :, 0:512],
                                qs[:].unsqueeze(1)
                                .broadcast_to([128, JCH, 512]),
                                op=OP.mult)
                            nc.vector.tensor_reduce(
                                L[:, ch * JCH:(ch + 1) * JCH, :],
                                prod[:].rearrange("p j (h d) -> p j h d", d=HD),
                                axis=AX.X, op=OP.add)
                        # logits -> softmax weights
                        bt = pool.tile([128, Db], F32, tag="bt")
                        nc.sync.dma_start(
                            bt[:],
                            biasfl[bias_off:bias_off + 128 * Db]
                            .rearrange("(p d) -> p d", p=128))
                        bias_off += 128 * Db
                        nc.vector.scalar_tensor_tensor(
                            L[:], L[:], SCALE,
                            bt[:].unsqueeze(2).broadcast_to([128, Db, H]),
                            op0=OP.mult, op1=OP.add)
                        P = pool.tile([128, Db, H], F32, tag="P")
                        nc.scalar.activation(P[:], L[:], AF.Exp)
                        dn = pool.tile([128, H], F32, tag="dn")
                        nc.vector.tensor_reduce(dn[:], P[:].transpose([0, 2, 1]),
                                                axis=AX.X, op=OP.add)
                        nc.vector.tensor_scalar_max(dn[:], dn[:], 1e-35)
                        rc = pool.tile([128, H], F32, tag="rc")
                        nc.vector.reciprocal(rc[:], dn[:])
                        nc.vector.tensor_tensor(
                            P[:], P[:],
                            rc[:].unsqueeze(1).broadcast_to([128, Db, H]),
                            op=OP.mult)
                        for cc in range(3):
                            tt = pool.tile([128, Db, H], F32, tag="tt")
                            nc.vector.tensor_tensor(
                                tt[:], P[:],
                                posb[:, :, cc].unsqueeze(2)
                                .broadcast_to([128, Db, H]),
                                op=OP.mult)
                            nc.vector.tensor_reduce(
                                A[:, :, cc], tt[:].transpose([0, 2, 1]),
                                axis=AX.X, op=OP.add)
                    else:
                        nc.vector.memset(A[:], 0.0)
                    # att -= pos
                    pp = pool.tile([128, 3], F32, tag="pp")
                    nc.sync.dma_start(pp[:], posp[b * 128:(b + 1) * 128, :])
                    nc.vector.tensor_tensor(
                        A[:], A[:],
                        pp[:].unsqueeze(1).broadcast_to([128, H, 3]),
                        op=OP.subtract)
                    # A^T padded into at5[:, sb, :]
                    Ab = pool.tile([128, 24], BF16, tag="Ab")
                    nc.scalar.activation(Ab[:], A[:].rearrange("p h c -> p (h c)"),
                                         AF.Copy)
                    atp = psp.tile([128, 128], BF16, tag="psB")
                    nc.tensor.transpose(atp[0:24, :], Ab[:], ident[:])
                    nc.vector.memset(at5[:, sb, :], 0.0)
                    nc.scalar.activation(at5[0:24, sb, :], atp[0:24, :], AF.Copy)

                # ---- MLPs for this 512-row group (transposed orientation)
                def rhs_z(ic):
                    return zTg[:, :, ic, :]

                def mlp_layer(win, nin, rhs_fn, nm, gelu=True):
                    ht = mp.tile([128, 8, 512], BF16, tag=nm)
                    for oc in range(8):
                        ps = psp.tile([128, 512], F32, tag="psA")
                        for ic in range(nin):
                            nc.tensor.matmul(ps[:], win[:, ic, oc * 128:(oc + 1) * 128],
                                             rhs_fn(ic), start=(ic == 0),
                                             stop=(ic == nin - 1))
                        if gelu:
                            nc.scalar.activation(ht[:, oc, :], ps[:],
                                                 AF.Gelu_apprx_tanh)
                        else:
                            nc.scalar.activation(ht[:, oc, :], ps[:], AF.Copy)
                    return ht

                # energy
                h1e = mlp_layer(wei, 4, rhs_z, "h1e")
                g2e = mlp_layer(weh, 8, lambda ic: h1e[:, ic, :], "g2e")
                h2e = mp.tile([128, 8, 512], BF16, tag="h2e")
                nc.vector.tensor_tensor(h2e[:], g2e[:], h1e[:], op=OP.add)
                pet = psp.tile([3, 512], F32, tag="psC")
                pe = pet[0:1, :]
                for ic in range(8):
                    nc.tensor.matmul(pe[:], weo[:, ic, :], h2e[:, ic, :],
                                     start=(ic == 0), stop=(ic == 7))
                es = pool.tile([1, 512], F32, tag="es")
                nc.scalar.activation(es[:], pe[:], AF.Copy)

                # forces
                def rhs_f(ic):
                    if ic < 4:
                        return rhs_z(ic)
                    return at5[:]

                h1f = mlp_layer(wfi, 5, rhs_f, "h1f")
                g2f = mlp_layer(wfh, 8, lambda ic: h1f[:, ic, :], "g2f")
                h2f = mp.tile([128, 8, 512], BF16, tag="h2f")
                nc.vector.tensor_tensor(h2f[:], g2f[:], h1f[:], op=OP.add)
                pf = psp.tile([3, 512], F32, tag="psC")
                for ic in range(8):
                    nc.tensor.matmul(pf[:], wfo[:, ic, :], h2f[:, ic, :],
                                     start=(ic == 0), stop=(ic == 7))
                fs = pool.tile([3, 512], F32, tag="fs")
                nc.scalar.activation(fs[:], pf[:], AF.Copy)
                nc.sync.dma_start(out[g, 0:1, :], es[:])
                nc.sync.dma_start(out[g, 1:4, :], fs[:])

    nc.compile()
    return nc


def _prep(inputs):
    """Host-side sharding: returns (in_maps, perms, dbs, total_chunks, totb)."""
    x = np.asarray(inputs["x"], np.float32)
    ei = np.asarray(inputs["edge_index"]).astype(np.int64)
    ab = np.asarray(inputs["att_bias"], np.float32)
    pos = np.asarray(inputs["pos"], np.float32)

    row, col = ei[0], ei[1]
    order = np.argsort(row, kind="stable")
    rs, cs, bs = row[order], col[order], ab[order]
    core_bounds = np.searchsorted(rs, np.arange(NCORES + 1) * RPC)

    # per-core degree & permutation
    degs, perms = [], []
    for m in range(NCORES):
        lo, hi = core_bounds[m], core_bounds[m + 1]
        deg = np.bincount(rs[lo:hi] - m * RPC, minlength=RPC)
        perm = np.argsort(-deg, kind="stable")
        degs.append(deg)
        perms.append(perm)

    # global (cross-core max) padded block degrees
    dbs = []
    for b in range(NBLK):
        mx = 0
        for m in range(NCORES):
            blk = perms[m][b * 128:(b + 1) * 128]
            if len(blk):
                mx = max(mx, int(degs[m][blk].max()) if b * 128 < RPC else 0)
        mx = max(mx, JCH)
        dbs.append(((mx + JCH - 1) // JCH) * JCH)
    dbs = tuple(dbs)
    total_chunks = sum(d // JCH for d in dbs)
    totb = 128 * sum(dbs)

    # folded weights (fp32 host math)
    g_att = np.asarray(inputs["g_att"], np.float32)
    b_att = np.asarray(inputs["b_att"], np.float32)
    g_mlp = np.asarray(inputs["g_mlp"], np.float32)
    b_mlp = np.asarray(inputs["b_mlp"], np.float32)
    Wq = np.asarray(inputs["Wq"], np.float32) * g_att[:, None]
    Wk = np.asarray(inputs["Wk"], np.float32) * g_att[:, None]
    # fold LN mean-subtract into Wk (phase 1 computes k from raw x)
    Wk = Wk - np.ones((DM, 1), np.float32) * (Wk.sum(axis=0, keepdims=True) / DM)
    Wei = np.asarray(inputs["We_in"], np.float32) * g_mlp[:, None]
    Wfi = np.asarray(inputs["Wf_in"], np.float32).copy()
    Wfi[:512] *= g_mlp[:, None]
    for nm, bsrc, wmat in [("bq", "bq", None)]:
        pass
    bias_names = ["bq", "bk", "be_in", "be_h", "be_out", "bf_in", "bf_h",
                  "bf_out"]
    allb = [np.asarray(inputs[n], np.float32) for n in bias_names]
    bq2 = allb[0] + b_att @ np.asarray(inputs["Wq"], np.float32)
    bk2 = allb[1] + b_att @ np.asarray(inputs["Wk"], np.float32)
    bei2 = allb[2] + b_mlp @ np.asarray(inputs["We_in"], np.float32)
    bfi2 = allb[5] + b_mlp @ np.asarray(inputs["Wf_in"], np.float32)[:512]
    assert all(np.abs(v).max() == 0 for v in
               [bq2, bk2, bei2, bfi2, allb[3], allb[4], allb[6], allb[7]]), \
        "nonzero biases not supported by this build"

    Weh = np.asarray(inputs["We_h"], np.float32)
    Weo = np.asarray(inputs["We_out"], np.float32)
    Wfh = np.asarray(inputs["Wf_h"], np.float32)
    Wfo = np.asarray(inputs["Wf_out"], np.float32)

    def wtile(W, nchunk):
        Wp = np.zeros((nchunk * 128, W.shape[1]), np.float32)
        Wp[:W.shape[0]] = W
        return Wp.reshape(nchunk, 128, W.shape[1]).astype(ml_dtypes.bfloat16)

    xf_np = np.zeros((TROWS, DM), np.float32)
    xf_np[:N] = x
    shared = {
        "xf": xf_np, "posf": pos,
        "ident": np.eye(128).astype(ml_dtypes.bfloat16),
        "wq": wtile(Wq, 4), "wk": wtile(Wk, 4),
        "wei": wtile(Wei, 4), "weh": wtile(Weh, 8), "weo": wtile(Weo, 8),
        "wfi": wtile(Wfi, 5), "wfh": wtile(Wfh, 8), "wfo": wtile(Wfo, 8),
    }

    in_maps = []
    for m in range(NCORES):
        lo = core_bounds[m]
        deg, perm = degs[m], perms[m]
        perm_pad = np.concatenate([perm, np.zeros(RPAD - RPC, np.int64)])
        deg_pad = np.concatenate([deg[perm], np.zeros(RPAD - RPC, np.int64)])
        # edge offsets into the sorted-per-core arrays
        roff = np.concatenate([[0], np.cumsum(deg)]) + lo

        colchunks = np.zeros((total_chunks, JCH * 128), np.int16)
        biasfl = np.full(totb, PADBIAS, np.float32)
        ci = 0
        boff = 0
        for b in range(NBLK):
            Db = dbs[b]
            cols_blk = np.zeros((128, Db), np.int64)
            bias_blk = np.full((128, Db), PADBIAS, np.float32)
            for p in range(128):
                r = perm_pad[b * 128 + p]
                d = int(deg_pad[b * 128 + p])
                if d:
                    e0 = roff[r]
                    cols_blk[p, :d] = cs[e0:e0 + d]
                    bias_blk[p, :d] = bs[e0:e0 + d]
            biasfl[boff:boff + 128 * Db] = bias_blk.reshape(-1)
            boff += 128 * Db
            for ch in range(Db // JCH):
                # gather order: position jj*128 + p
                colchunks[ci] = cols_blk[:, ch * JCH:(ch + 1) * JCH].T.reshape(-1)
                ci += 1
        # wrap indices: idx i -> [i%16, i//16], replicate x8
        cw = colchunks.reshape(total_chunks, JCH * 128 // 16, 16)
        cw = np.ascontiguousarray(np.transpose(cw, (0, 2, 1)))
        cidx = np.tile(cw, (1, 8, 1)).astype(np.int16)

        im = dict(shared)
        im["xp"] = x[m * RPC:(m + 1) * RPC][perm_pad % RPC]
        im["posp"] = pos[m * RPC:(m + 1) * RPC][perm_pad % RPC]
        im["colidx"] = cidx
        im["biasfl"] = biasfl
        in_maps.append(im)
    return in_maps, perms, dbs, total_chunks, totb


def kernel(**inputs):
    in_maps, perms, dbs, total_chunks, totb = _prep(inputs)
    nc = _build(dbs, total_chunks, totb)
    res = bass_utils.run_bass_kernel_spmd(
        nc, in_maps, core_ids=list(range(NCORES)))
    energy = np.zeros((N, 1), np.float32)
    forces = np.zeros((N, 3), np.float32)
    for m in range(NCORES):
        o = res.results[m]["out"]              # [NGRP, 4, 512]
        e_perm = o[:, 0, :].reshape(RPAD)
        f_perm = np.moveaxis(o[:, 1:4, :], 1, 2).reshape(RPAD, 3)
        gi = m * RPC + perms[m]
        energy[gi, 0] = e_perm[:RPC]
        forces[gi] = f_perm[:RPC]
    return energy, forces


# revision 15
# speedup vs baseline: 1.2386x; 1.2235x over previous
"""Trainium2 Bass kernel for nn_AttentionOutputModule (sparse attention + MLPs).

Sharding: 8 cores, each owns 2500 destination rows; edges partitioned by
destination row. Per core, rows are sorted by degree and grouped into 128-row
blocks; each block's edge lists are padded to the block max degree. k (for all
20000 nodes) + pos are written to a DRAM table, and per-edge rows are fetched
with dma_gather. Softmax is computed without max-subtraction (logits are
bounded, mathematically identical). MLPs run in a transposed orientation
(dims on partitions) so no per-layer activation transposes are needed.
"""
import functools
import numpy as np
import ml_dtypes

import concourse.bacc as bacc
import concourse.mybir as mybir
from concourse import tile, bass_utils

# problem dims (hardcoded per contract)
N, DM, H, HD, HID, FIN = 20000, 512, 8, 64, 1024, 536
NCORES = 8
RPC = N // NCORES          # 2500 rows per core
RPAD = 2560                # padded to 20 blocks of 128
NBLK = RPAD // 128         # 20
GRP = 4                    # blocks per MLP group (512 rows)
NGRP = NBLK // GRP         # 5
JCH = 8                    # j's per gather chunk -> 1024 idxs (SWDGE ring limit)
TB = (N + 127) // 128      # 157 x_full blocks
TROWS = TB * 128           # 20096
TC = 640                   # bf16 gather table cols: k(512)bf16 | pos(3)f32-bits | pad
SCALE = 1.0 / float(np.sqrt(HD))
EPS = 1e-5
PADBIAS = -30000.0

F32 = mybir.dt.float32
BF16 = mybir.dt.bfloat16
I16 = mybir.dt.int16
AF = mybir.ActivationFunctionType
OP = mybir.AluOpType
AX = mybir.AxisListType
GELU = AF.Gelu_apprx_tanh


def _layernorm_block(nc, pool, xt, tag):
    """LN_raw of [128, 512] fp32 tile -> bf16 tile (affine folded into weights)."""
    s1 = pool.tile([128, 1], F32, tag=f"{tag}_s1")
    nc.vector.tensor_reduce(s1[:], xt[:], axis=AX.X, op=OP.add)
    sq = pool.tile([128, DM], F32, tag=f"{tag}_sq")
    s2 = pool.tile([128, 1], F32, tag=f"{tag}_s2")
    nc.scalar.activation(sq[:], xt, AF.Square, accum_out=s2[:])
    m = pool.tile([128, 1], F32, tag=f"{tag}_m")
    nc.vector.tensor_scalar_mul(m[:], s1[:], 1.0 / DM)
    m2 = pool.tile([128, 1], F32, tag=f"{tag}_m2")
    nc.vector.tensor_tensor(m2[:], m[:], m[:], op=OP.mult)
    t2 = pool.tile([128, 1], F32, tag=f"{tag}_t2")
    nc.vector.tensor_scalar(t2[:], s2[:], 1.0 / DM, EPS, op0=OP.mult, op1=OP.add)
    var = pool.tile([128, 1], F32, tag=f"{tag}_var")
    nc.vector.tensor_tensor(var[:], t2[:], m2[:], op=OP.subtract)
    sd = pool.tile([128, 1], F32, tag=f"{tag}_sd")
    nc.scalar.activation(sd[:], var[:], AF.Sqrt)
    inv = pool.tile([128, 1], F32, tag=f"{tag}_inv")
    nc.vector.reciprocal(inv[:], sd[:])
    zb = pool.tile([128, DM], BF16, tag=f"{tag}_zb")
    nc.vector.tensor_scalar(zb[:], xt, m[:], inv[:],
                            op0=OP.subtract, op1=OP.mult)
    return zb


def _transpose128(nc, pool, psp, src_ap, ident, tag, out_dtype=BF16):
    """PE-transpose a [128,128] bf16 AP -> sbuf tile [128,128]."""
    tp = psp.tile([128, 128], BF16, tag="psB")
    nc.tensor.transpose(tp[:], src_ap, ident[:])
    ts = pool.tile([128, 128], out_dtype, tag=f"{tag}_sb")
    nc.scalar.activation(ts[:], tp[:], AF.Copy)
    return ts


@functools.lru_cache(maxsize=2)
def _build(dbs: tuple, total_chunks: int, totb: int):
    """Build + compile the SPMD program. dbs: padded per-block degrees (len 20)."""
    nc = bacc.Bacc("TRN2", target_bir_lowering=False, debug=False,
                   num_devices=NCORES)

    def inp(nm, sh, dt=F32):
        return nc.dram_tensor(nm, sh, dt, kind="ExternalInput").ap()

    xf = inp("xf", [TROWS, DM])
    xp = inp("xp", [RPAD, DM])
    posf = inp("posf", [N, 3])
    posp = inp("posp", [RPAD, 3])
    colidx = inp("colidx", [total_chunks, 128, JCH * 128 // 16], I16)
    biasfl = inp("biasfl", [totb])
    ident_in = inp("ident", [128, 128], BF16)
    wq_in = inp("wq", [4, 128, DM], BF16)
    wk_in = inp("wk", [4, 128, DM], BF16)
    wei_in = inp("wei", [4, 128, HID], BF16)
    weh_in = inp("weh", [8, 128, HID], BF16)
    weo_in = inp("weo", [8, 128, 1], BF16)
    wfi_in = inp("wfi", [5, 128, HID], BF16)
    wfh_in = inp("wfh", [8, 128, HID], BF16)
    wfo_in = inp("wfo", [8, 128, 3], BF16)
    out = nc.dram_tensor("out", [NGRP, 4, 512], F32, kind="ExternalOutput").ap()

    with tile.TileContext(nc) as tc:
        with tc.tile_pool(name="dram", bufs=1, space="DRAM") as dpool, \
             tc.tile_pool(name="wpool", bufs=1) as wp, \
             tc.tile_pool(name="pool", bufs=2) as pool, \
             tc.tile_pool(name="psum", bufs=2, space="PSUM") as psp:

            ktab = dpool.tile([TROWS, TC], BF16)

            # resident weights
            def load_w(ap_in, nchunk, ncol, nm):
                t = wp.tile([128, nchunk, ncol], BF16, tag=nm)
                nc.sync.dma_start(t[:], ap_in.transpose([1, 0, 2]))
                return t

            wq = load_w(wq_in, 4, DM, "wq")
            wei = load_w(wei_in, 4, HID, "wei")
            weh = load_w(weh_in, 8, HID, "weh")
            weo = load_w(weo_in, 8, 1, "weo")
            wfi = load_w(wfi_in, 5, HID, "wfi")
            wfh = load_w(wfh_in, 8, HID, "wfh")
            wfo = load_w(wfo_in, 8, 3, "wfo")
            ident = wp.tile([128, 128], BF16, tag="ident")
            nc.sync.dma_start(ident[:], ident_in)

            # ---- phase 1: k table (all 20096 rows, replicated on each core)
            nc.sync.dma_start(ktab[:, :].bitcast(F32)[0:N, 256:259], posf)
            p1ctx = tc.tile_pool(name="p1pool", bufs=2)
            p1p = p1ctx.__enter__()
            wk = p1p.tile([128, 4, DM], BF16, tag="wk")
            nc.sync.dma_start(wk[:], wk_in.transpose([1, 0, 2]))
            identf = p1p.tile([128, 128], F32, tag="identf")
            nc.vector.tensor_copy(identf[:], ident[:])
            for tb4 in range((TB + 3) // 4):
              nblk4 = min(4, TB - tb4 * 4)
              xt4 = p1p.tile([128, 4, DM], F32, tag="p1_x4")
              nc.sync.dma_start(
                  xt4[:, 0:nblk4, :],
                  xf[tb4 * 512:tb4 * 512 + nblk4 * 128, :]
                  .rearrange("(i p) d -> p i d", p=128))
              ks4 = p1p.tile([128, 4, DM], BF16, tag="p1_ks4")
              for i in range(nblk4):
                tb = tb4 * 4 + i
                xt = xt4[:, i, :]
                # LN stats only (mean folded into wk'' on host)
                s1 = p1p.tile([128, 1], F32, tag="p1_s1")
                nc.vector.tensor_reduce(s1[:], xt, axis=AX.X, op=OP.add)
                sq = p1p.tile([128, DM], F32, tag="p1_sq")
                s2 = p1p.tile([128, 1], F32, tag="p1_s2")
                nc.scalar.activation(sq[:], xt, AF.Square, accum_out=s2[:])
                m = p1p.tile([128, 1], F32, tag="p1_m")
                nc.vector.tensor_scalar_mul(m[:], s1[:], 1.0 / DM)
                m2 = p1p.tile([128, 1], F32, tag="p1_m2")
                nc.vector.tensor_tensor(m2[:], m[:], m[:], op=OP.mult)
                t2 = p1p.tile([128, 1], F32, tag="p1_t2")
                nc.vector.tensor_scalar(t2[:], s2[:], 1.0 / DM, EPS,
                                        op0=OP.mult, op1=OP.add)
                var = p1p.tile([128, 1], F32, tag="p1_var")
                nc.vector.tensor_tensor(var[:], t2[:], m2[:], op=OP.subtract)
                sd = p1p.tile([128, 1], F32, tag="p1_sd")
                nc.scalar.activation(sd[:], var[:], AF.Sqrt)
                inv = p1p.tile([128, 1], F32, tag="p1_inv")
                nc.vector.reciprocal(inv[:], sd[:])
                kp = psp.tile([128, DM], F32, tag="psA")
                for c in range(4):
                    tp = psp.tile([128, 128], F32, tag="psB")
                    nc.tensor.transpose(tp[:], xt[:, c * 128:(c + 1) * 128],
                                        identf[:])
                    xT = p1p.tile([128, 128], BF16, tag="p1_xT")
                    nc.any.tensor_copy(xT[:], tp[:])
                    nc.tensor.matmul(kp[:], xT[:], wk[:, c, :],
                                     start=(c == 0), stop=(c == 3))
                nc.scalar.activation(ks4[:, i, :], kp[:], AF.Copy, scale=inv[:])
              nc.sync.dma_start(
                  ktab[tb4 * 512:tb4 * 512 + nblk4 * 128, 0:512]
                  .rearrange("(i p) d -> p i d", p=128),
                  ks4[:, 0:nblk4, :])

            p1ctx.__exit__(None, None, None)

            # ---- phase 2+3: per 512-row group
            p2stk = [tc.tile_pool(name="gpool", bufs=2),
                     tc.tile_pool(name="xpool", bufs=1),
                     tc.tile_pool(name="ppool", bufs=2),
                     tc.tile_pool(name="zpool", bufs=1),
                     tc.tile_pool(name="mpool", bufs=1)]
            gp, xpl, ppl, zpl, mp = [c.__enter__() for c in p2stk]
            chunk_ctr = 0
            bias_off = 0
            for g in range(NGRP):
                zTg = zpl.tile([128, GRP, 4, 128], BF16, tag="zTg")
                at5 = zpl.tile([128, GRP, 128], BF16, tag="at5")
                xt4 = xpl.tile([128, GRP, DM], F32, tag="p2_x4")
                nc.sync.dma_start(
                    xt4[:], xp[g * 512:(g + 1) * 512, :]
                    .rearrange("(i p) d -> p i d", p=128))
                pp4 = pool.tile([128, GRP, 3], F32, tag="pp4")
                nc.sync.dma_start(
                    pp4[:], posp[g * 512:(g + 1) * 512, :]
                    .rearrange("(i p) d -> p i d", p=128))
                for sb in range(GRP):
                    b = g * GRP + sb
                    Db = dbs[b]
                    zb = _layernorm_block(nc, pool, xt4[:, sb, :], "p2")
                    for c in range(4):
                        tp = psp.tile([128, 128], BF16, tag="psB")
                        nc.tensor.transpose(tp[:], zb[:, c * 128:(c + 1) * 128],
                                            ident[:])
                        nc.scalar.activation(zTg[:, sb, c, :], tp[:], AF.Copy)
                    # q = z @ Wq  [128 rows, 512] fp32
                    qp = psp.tile([128, DM], F32, tag="psA")
                    for c in range(4):
                        nc.tensor.matmul(qp[:], zTg[:, sb, c, :], wq[:, c, :],
                                         start=(c == 0), stop=(c == 3))
                    qs = pool.tile([128, DM], BF16, tag="qs")
                    nc.scalar.activation(qs[:], qp[:], AF.Copy)

                    # attention
                    A = pool.tile([128, H, 3], F32, tag="A")
                    if Db > 0:
                        L = pool.tile([128, Db, H], F32, tag="L")
                        posb = pool.tile([128, Db, 3], F32, tag="posb")
                        nchunk = Db // JCH
                        IW = JCH * 128 // 16
                        idxb = gp.tile([128, nchunk, IW], I16, tag="idxb")
                        nc.sync.dma_start(
                            idxb[:],
                            colidx[chunk_ctr:chunk_ctr + nchunk]
                            .transpose([1, 0, 2]))
                        chunk_ctr += nchunk
                        for ch in range(nchunk):
                            G = gp.tile([128, JCH, TC], BF16, tag="G")
                            nc.gpsimd.dma_gather(
                                G[:], ktab[:], idxb[:, ch, :],
                                num_idxs=JCH * 128, num_idxs_reg=JCH * 128,
                                elem_size=TC)
                            nc.vector.tensor_copy(
                                posb[:, ch * JCH:(ch + 1) * JCH, :],
                                G[:, :, 512:518].bitcast(F32))
                            for hf in range(2):
                                prod = ppl.tile([128, JCH // 2, 512], BF16,
                                                tag="prod")
                                nc.vector.tensor_tensor(
                                    prod[:],
                                    G[:, hf * (JCH // 2):(hf + 1) * (JCH // 2),
                                      0:512],
                                    qs[:].unsqueeze(1)
                                    .broadcast_to([128, JCH // 2, 512]),
                                    op=OP.mult)
                                nc.vector.tensor_reduce(
                                    L[:, ch * JCH + hf * (JCH // 2):
                                      ch * JCH + (hf + 1) * (JCH // 2), :],
                                    prod[:].rearrange("p j (h d) -> p j h d",
                                                      d=HD),
                                    axis=AX.X, op=OP.add)
                        # logits -> softmax weights
                        bt = pool.tile([128, Db], F32, tag="bt")
                        nc.sync.dma_start(
                            bt[:],
                            biasfl[bias_off:bias_off + 128 * Db]
                            .rearrange("(p d) -> p d", p=128))
                        bias_off += 128 * Db
                        nc.vector.scalar_tensor_tensor(
                            L[:], L[:], SCALE,
                            bt[:].unsqueeze(2).broadcast_to([128, Db, H]),
                            op0=OP.mult, op1=OP.add)
                        P = pool.tile([128, Db, H], F32, tag="P")
                        nc.scalar.activation(P[:], L[:], AF.Exp)
                        dn = pool.tile([128, H], F32, tag="dn")
                        nc.vector.tensor_reduce(dn[:], P[:].transpose([0, 2, 1]),
                                                axis=AX.X, op=OP.add)
                        nc.vector.tensor_scalar_max(dn[:], dn[:], 1e-35)
                        rc = pool.tile([128, H], F32, tag="rc")
                        nc.vector.reciprocal(rc[:], dn[:])
                        nc.vector.tensor_tensor(
                            P[:], P[:],
                            rc[:].unsqueeze(1).broadcast_to([128, Db, H]),
                            op=OP.mult)
                        for cc in range(3):
                            tt = pool.tile([128, Db, H], F32, tag="tt")
                            nc.vector.tensor_tensor(
                                tt[:], P[:],
                                posb[:, :, cc].unsqueeze(2)
                                .broadcast_to([128, Db, H]),
                                op=OP.mult)
                            nc.vector.tensor_reduce(
                                A[:, :, cc], tt[:].transpose([0, 2, 1]),
                                axis=AX.X, op=OP.add)
                    else:
                        nc.vector.memset(A[:], 0.0)
                    # att -= pos
                    nc.vector.tensor_tensor(
                        A[:], A[:],
                        pp4[:, sb, :].unsqueeze(1).broadcast_to([128, H, 3]),
                        op=OP.subtract)
                    # A^T padded into at5[:, sb, :]
                    Ab = pool.tile([128, 24], BF16, tag="Ab")
                    nc.scalar.activation(Ab[:], A[:].rearrange("p h c -> p (h c)"),
                                         AF.Copy)
                    atp = psp.tile([128, 128], BF16, tag="psB")
                    nc.tensor.transpose(atp[0:24, :], Ab[:], ident[:])
                    nc.vector.memset(at5[:, sb, :], 0.0)
                    nc.scalar.activation(at5[0:24, sb, :], atp[0:24, :], AF.Copy)

                # ---- MLPs for this 512-row group (transposed orientation)
                def rhs_z(ic):
                    return zTg[:, :, ic, :]

                def mlp_layer(win, nin, rhs_fn, nm, gelu=True):
                    ht = mp.tile([128, 8, 512], BF16, tag=nm)
                    for oc in range(8):
                        ps = psp.tile([128, 512], F32, tag="psA")
                        for ic in range(nin):
                            nc.tensor.matmul(ps[:], win[:, ic, oc * 128:(oc + 1) * 128],
                                             rhs_fn(ic), start=(ic == 0),
                                             stop=(ic == nin - 1))
                        if gelu:
                            nc.scalar.activation(ht[:, oc, :], ps[:],
                                                 GELU)
                        else:
                            nc.scalar.activation(ht[:, oc, :], ps[:], AF.Copy)
                    return ht

                # energy
                h1e = mlp_layer(wei, 4, rhs_z, "h1e")
                g2e = mlp_layer(weh, 8, lambda ic: h1e[:, ic, :], "g2e")
                nc.vector.tensor_tensor(g2e[:], g2e[:], h1e[:], op=OP.add)
                h2e = g2e
                pet = psp.tile([3, 512], F32, tag="psC")
                pe = pet[0:1, :]
                for ic in range(8):
                    nc.tensor.matmul(pe[:], weo[:, ic, :], h2e[:, ic, :],
                                     start=(ic == 0), stop=(ic == 7))
                es = pool.tile([1, 512], F32, tag="es")
                nc.scalar.activation(es[:], pe[:], AF.Copy)

                # forces
                def rhs_f(ic):
                    if ic < 4:
                        return rhs_z(ic)
                    return at5[:]

                h1f = mlp_layer(wfi, 5, rhs_f, "h1f")
                g2f = mlp_layer(wfh, 8, lambda ic: h1f[:, ic, :], "g2f")
                nc.vector.tensor_tensor(g2f[:], g2f[:], h1f[:], op=OP.add)
                h2f = g2f
                pf = psp.tile([3, 512], F32, tag="psC")
                for ic in range(8):
                    nc.tensor.matmul(pf[:], wfo[:, ic, :], h2f[:, ic, :],
                                     start=(ic == 0), stop=(ic == 7))
                fs = pool.tile([3, 512], F32, tag="fs")
                nc.scalar.activation(fs[:], pf[:], AF.Copy)
                nc.sync.dma_start(out[g, 0:1, :], es[:])
                nc.sync.dma_start(out[g, 1:4, :], fs[:])
            for c in reversed(p2stk):
                c.__exit__(None, None, None)

    nc.compile()
    return nc


def _prep(inputs):
    """Host-side sharding: returns (in_maps, perms, dbs, total_chunks, totb)."""
    x = np.asarray(inputs["x"], np.float32)
    ei = np.asarray(inputs["edge_index"]).astype(np.int64)
    ab = np.asarray(inputs["att_bias"], np.float32)
    pos = np.asarray(inputs["pos"], np.float32)

    row, col = ei[0], ei[1]
    order = np.argsort(row, kind="stable")
    rs, cs, bs = row[order], col[order], ab[order]
    core_bounds = np.searchsorted(rs, np.arange(NCORES + 1) * RPC)

    # per-core degree & permutation
    degs, perms = [], []
    for m in range(NCORES):
        lo, hi = core_bounds[m], core_bounds[m + 1]
        deg = np.bincount(rs[lo:hi] - m * RPC, minlength=RPC)
        perm = np.argsort(-deg, kind="stable")
        degs.append(deg)
        perms.append(perm)

    # global (cross-core max) padded block degrees
    dbs = []
    for b in range(NBLK):
        mx = 0
        for m in range(NCORES):
            blk = perms[m][b * 128:(b + 1) * 128]
            if len(blk):
                mx = max(mx, int(degs[m][blk].max()) if b * 128 < RPC else 0)
        mx = max(mx, JCH)
        dbs.append(((mx + JCH - 1) // JCH) * JCH)
    dbs = tuple(dbs)
    total_chunks = sum(d // JCH for d in dbs)
    totb = 128 * sum(dbs)

    # folded weights (fp32 host math)
    g_att = np.asarray(inputs["g_att"], np.float32)
    b_att = np.asarray(inputs["b_att"], np.float32)
    g_mlp = np.asarray(inputs["g_mlp"], np.float32)
    b_mlp = np.asarray(inputs["b_mlp"], np.float32)
    Wq = np.asarray(inputs["Wq"], np.float32) * g_att[:, None]
    Wk = np.asarray(inputs["Wk"], np.float32) * g_att[:, None]
    # fold LN mean-subtract into Wk (phase 1 computes k from raw x)
    Wk = Wk - np.ones((DM, 1), np.float32) * (Wk.sum(axis=0, keepdims=True) / DM)
    Wei = np.asarray(inputs["We_in"], np.float32) * g_mlp[:, None]
    Wfi = np.asarray(inputs["Wf_in"], np.float32).copy()
    Wfi[:512] *= g_mlp[:, None]
    for nm, bsrc, wmat in [("bq", "bq", None)]:
        pass
    bias_names = ["bq", "bk", "be_in", "be_h", "be_out", "bf_in", "bf_h",
                  "bf_out"]
    allb = [np.asarray(inputs[n], np.float32) for n in bias_names]
    bq2 = allb[0] + b_att @ np.asarray(inputs["Wq"], np.float32)
    bk2 = allb[1] + b_att @ np.asarray(inputs["Wk"], np.float32)
    bei2 = allb[2] + b_mlp @ np.asarray(inputs["We_in"], np.float32)
    bfi2 = allb[5] + b_mlp @ np.asarray(inputs["Wf_in"], np.float32)[:512]
    assert all(np.abs(v).max() == 0 for v in
               [bq2, bk2, bei2, bfi2, allb[3], allb[4], allb[6], allb[7]]), \
        "nonzero biases not supported by this build"

    Weh = np.asarray(inputs["We_h"], np.float32)
    Weo = np.asarray(inputs["We_out"], np.float32)
    Wfh = np.asarray(inputs["Wf_h"], np.float32)
    Wfo = np.asarray(inputs["Wf_out"], np.float32)

    def wtile(W, nchunk):
        Wp = np.zeros((nchunk * 128, W.shape[1]), np.float32)
        Wp[:W.shape[0]] = W
        return Wp.reshape(nchunk, 128, W.shape[1]).astype(ml_dtypes.bfloat16)

    xf_np = np.zeros((TROWS, DM), np.float32)
    xf_np[:N] = x
    shared = {
        "xf": xf_np, "posf": pos,
        "ident": np.eye(128).astype(ml_dtypes.bfloat16),
        "wq": wtile(Wq, 4), "wk": wtile(Wk, 4),
        "wei": wtile(Wei, 4), "weh": wtile(Weh, 8), "weo": wtile(Weo, 8),
        "wfi": wtile(Wfi, 5), "wfh": wtile(Wfh, 8), "wfo": wtile(Wfo, 8),
    }

    in_maps = []
    for m in range(NCORES):
        lo = core_bounds[m]
        deg, perm = degs[m], perms[m]
        perm_pad = np.concatenate([perm, np.zeros(RPAD - RPC, np.int64)])
        deg_pad = np.concatenate([deg[perm], np.zeros(RPAD - RPC, np.int64)])
        # edge offsets into the sorted-per-core arrays
        roff = np.concatenate([[0], np.cumsum(deg)]) + lo

        colchunks = np.zeros((total_chunks, JCH * 128), np.int16)
        biasfl = np.full(totb, PADBIAS, np.float32)
        ci = 0
        boff = 0
        for b in range(NBLK):
            Db = dbs[b]
            cols_blk = np.zeros((128, Db), np.int64)
            bias_blk = np.full((128, Db), PADBIAS, np.float32)
            for p in range(128):
                r = perm_pad[b * 128 + p]
                d = int(deg_pad[b * 128 + p])
                if d:
                    e0 = roff[r]
                    cols_blk[p, :d] = cs[e0:e0 + d]
                    bias_blk[p, :d] = bs[e0:e0 + d]
            biasfl[boff:boff + 128 * Db] = bias_blk.reshape(-1)
            boff += 128 * Db
            for ch in range(Db // JCH):
                # gather order: position jj*128 + p
                colchunks[ci] = cols_blk[:, ch * JCH:(ch + 1) * JCH].T.reshape(-1)
                ci += 1
        # wrap indices: idx i -> [i%16, i//16], replicate x8
        cw = colchunks.reshape(total_chunks, JCH * 128 // 16, 16)
        cw = np.ascontiguousarray(np.transpose(cw, (0, 2, 1)))
        cidx = np.tile(cw, (1, 8, 1)).astype(np.int16)

        im = dict(shared)
        im["xp"] = x[m * RPC:(m + 1) * RPC][perm_pad % RPC]
        im["posp"] = pos[m * RPC:(m + 1) * RPC][perm_pad % RPC]
        im["colidx"] = cidx
        im["biasfl"] = biasfl
        in_maps.append(im)
    return in_maps, perms, dbs, total_chunks, totb


def kernel(**inputs):
    in_maps, perms, dbs, total_chunks, totb = _prep(inputs)
    nc = _build(dbs, total_chunks, totb)
    res = bass_utils.run_bass_kernel_spmd(
        nc, in_maps, core_ids=list(range(NCORES)))
    energy = np.zeros((N, 1), np.float32)
    forces = np.zeros((N, 3), np.float32)
    for m in range(NCORES):
        o = res.results[m]["out"]              # [NGRP, 4, 512]
        e_perm = o[:, 0, :].reshape(RPAD)
        f_perm = np.moveaxis(o[:, 1:4, :], 1, 2).reshape(RPAD, 3)
        gi = m * RPC + perms[m]
        energy[gi, 0] = e_perm[:RPC]
        forces[gi] = f_perm[:RPC]
    return energy, forces
